# revision 29
# baseline (speedup 1.0000x reference)
"""Trainium2 Bass kernel for a Neural ODE (tanh-MLP vector field, Heun/RK2).

Reference computation (per batch row y of width D=512):
    f(y) = tanh(y @ W1 + b1) @ W2 + b2          (H = 2048)
    10 Heun steps, dt = 0.1:
        k1 = f(y); k2 = f(y + dt*k1); y <- y + dt/2*(k1 + k2)

Sharding: data-parallel over the batch axis across 8 NeuronCores
(y0 [8192,512] -> 8 x [1024,512]); weights replicated.

Per-core layout: the state lives TRANSPOSED (y.T, [D, B_local] with D on
partitions) so both matmuls of the MLP chain need no on-chip transposes:
    h.T = W1.T @ y.T   (lhsT = W1 [K=D, M=H],  rhs = y.T  [K=D, N=B])
    z.T = W2.T @ ht.T  (lhsT = W2 [K=H, M=D],  rhs = ht.T [K=H, N=B])

All matmuls run in fp8-e4m3 with perf_mode=DoubleRow (contraction 256 per
pass, ~1.8x the bf16/f32r rate). Precision is recovered two ways:
  1. The ODE state Y = SY*y and the Heun updates stay fp32; only the
     matmul operands (Y8, Ymid8, tanh output) are fp8, with power-of-two
     tensor scales (SY, S1, S2) folded into activation scale / update
     scalars so quantization sits in e4m3's sweet spot.
  2. The dominant error source -- the *systematic* weight quantization
     error integrated over all 20 vector-field evals -- is suppressed by
     keeping KW=4 residual-compensated fp8 copies of each weight matrix
     (copy i rounds (i+1)*W - sum(copies[:i]), so their running mean
     tracks W to ~1/KW of one rounding error) and rotating through them
     across evals.  Measured end-to-end rel-l2 vs the fp32 reference:
     8.8e-3 (vs 2.3e-2 with a single RNE fp8 copy).

Scheduling: the two 512-wide batch chunks run as independent interleaved
integrations (sub_feval per chunk), so while one chunk's post-matmul DVE
ops run, the PE executes the other chunk's whole sub-feval -- feval-
boundary stalls vanish and HAM stays warm.  Within a sub-feval, layer-1
m-tiles are paired into 2-bank PSUM tiles (one 1024-elem tanh ACT per
pair, 3-deep pool to absorb ScalarE lag); layer 2 uses single-bank PSUM
tiles.  LDWEIGHTS (~184ns) hides under the 216ns matmul stream via the
PE background weight buffer (walrus ldw-opt rejects DoubleRow and is not
used).  Measured: 2560 matmuls at the 216ns N=512 issue rate, ~98% of
the 157 TF/s fp8 peak; HW exec ~576us on 8 cores.
"""

import numpy as np
import ml_dtypes

import concourse.bacc as bacc
import concourse.mybir as mybir
import concourse.tile as tile
from concourse.bass_utils import run_bass_kernel_spmd

N_CORES = 8
BATCH, D, H = 8192, 512, 2048
B = BATCH // N_CORES          # local batch per core: 1024
DT = 0.1
N_STEPS = 10
P = 128
F32 = mybir.dt.float32
F8 = mybir.dt.float8e4
U8 = mybir.dt.uint8

D_T = D // P                  # 4  d-tiles (128-row feature blocks)
H_T = H // P                  # 16 h-tiles
KT1 = D // 256                # 2  DoubleRow k-tiles, layer 1
KT2 = H // 256                # 8  DoubleRow k-tiles, layer 2
NCHUNK = 2                    # batch chunks per core (N=512 per matmul)
NW = B // NCHUNK              # 512

KW = 4                        # rotated fp8 weight copies
SY = 16.0                     # state scale   (Y = SY*y)
S1 = 256.0                    # W1 scale
S2 = 1024.0                   # W2 scale
DR = mybir.MatmulPerfMode.DoubleRow

_NC_CACHE = {}


def _build(has_bias):
    nc = bacc.Bacc("TRN2", target_bir_lowering=False, debug=False)
    # Host-side prep: y0t = SY * y0_shard.T (fp32); y0q = e4m3(SY*y0.T) bits;
    # w1q/w2q = KW residual-compensated e4m3 copies of S1*W1 / S2*W2 (bits);
    # b2s = S2*b2.
    y0t = nc.dram_tensor("y0t", [P, D_T * B], F32,
                         kind="ExternalInput").ap()
    y0q = nc.dram_tensor("y0q", [P, D_T * B], U8,
                         kind="ExternalInput").ap()
    w1q = nc.dram_tensor("w1q", [KW, P, KT1 * 2 * H], U8,
                         kind="ExternalInput").ap()
    b1 = nc.dram_tensor("b1", [P, H_T], F32, kind="ExternalInput").ap()
    w2q = nc.dram_tensor("w2q", [KW, P, KT2 * 2 * D], U8,
                         kind="ExternalInput").ap()
    b2s = nc.dram_tensor("b2s", [P, D_T], F32,
                         kind="ExternalInput").ap()
    outt = nc.dram_tensor("outt", [D, B], F32, kind="ExternalOutput").ap()

    TANH = mybir.ActivationFunctionType.Tanh
    MULT = mybir.AluOpType.mult
    ADD = mybir.AluOpType.add
    ASCALE = 1.0 / (S1 * SY)         # PSUM1 -> pre-activation
    C1 = DT * SY / S2                # Ymid = Y + C1*pzb
    CH = 0.5 * DT * SY / S2          # Yacc/Ynew = ... + CH*pzb

    with tile.TileContext(nc) as tc:
        with (
            tc.tile_pool(name="persist", bufs=1) as persist,
            tc.tile_pool(name="ps_h", bufs=3, space="PSUM") as ps_h_pool,
            tc.tile_pool(name="ps_z", bufs=2, space="PSUM") as ps_z_pool,
        ):
            # Persistent SBUF residents (per-partition bytes in parens).
            w1_sb = [persist.tile([P, KT1, 2, H], F8, tag=f"w1_{i}",
                                  name=f"w1_{i}") for i in range(KW)]  # 8K x4
            w2_sb = [persist.tile([P, KT2, 2, D], F8, tag=f"w2_{i}",
                                  name=f"w2_{i}") for i in range(KW)]  # 8K x4
            b1_sb = persist.tile([P, H_T], F32, tag="b1")
            b2_sb = persist.tile([P, D_T], F32, tag="b2")
            y_sb = persist.tile([P, D_T, NCHUNK, NW], F32, tag="y")      # 16K
            y_acc = persist.tile([P, D_T, NCHUNK, NW], F32, tag="yacc")  # 16K
            y8 = persist.tile([P, D_T, NCHUNK, NW], F8, tag="y8")        # 4K
            ym8 = persist.tile([P, D_T, NCHUNK, NW], F8, tag="ym8")      # 4K
            ht8 = persist.tile([P, H_T, NCHUNK, NW], F8, tag="ht8")      # 16K

            MP = 2                # m-tiles per PSUM tile (2 banks)

            # --- input DMAs, in consumption order; all host-prepacked
            # partition-major so each is one full-bandwidth transfer
            # (a 1MB weight copy lands in ~600ns vs ~6us for a scatter) ---
            nc.sync.dma_start(y8[:], y0q.bitcast(F8))
            nc.sync.dma_start(w1_sb[0][:], w1q[0].bitcast(F8))
            nc.sync.dma_start(w2_sb[0][:], w2q[0].bitcast(F8))
            nc.sync.dma_start(y_sb[:], y0t[:])
            nc.sync.dma_start(b1_sb[:], b1[:])
            nc.sync.dma_start(b2_sb[:], b2s[:])
            for i in range(1, KW):
                nc.sync.dma_start(w1_sb[i][:], w1q[i].bitcast(F8))
                nc.sync.dma_start(w2_sb[i][:], w2q[i].bitcast(F8))

            def sub_feval(X8, wi, c, consume):
                """Vector-field eval for batch chunk c on fp8 state X8.

                The two chunks run as independent interleaved integrations:
                while this chunk's boundary consume runs on the DVE, the PE
                executes the other chunk's whole sub-feval, so feval-
                boundary stalls vanish by construction.

                Layer 1: psum = W1q.T @ X8 (DoubleRow, K=256/pass); one
                1024-elem ACT per m-pair (2-bank PSUM tile) writes
                ht8 = e4m3(tanh(psum/(S1*SY) + b1)).
                Layer 2: pz = W2q.T @ ht8 per dm (single-bank PSUM);
                consume(c, dm, pz) handles the S2-scaled vector-field
                tile [P, NW].
                """
                for mp in range(H_T // MP):
                    ph = ps_h_pool.tile([P, MP, NW], F32, tag="ps_h",
                                        name="ph")
                    for mm in range(MP):
                        m = mp * MP + mm
                        for t in range(KT1):
                            nc.tensor.matmul(
                                ph[:, mm, :],
                                w1_sb[wi][:, t, :, m * P:(m + 1) * P],
                                X8[:, 2 * t:2 * t + 2, c, :],
                                start=(t == 0), stop=(t == KT1 - 1),
                                perf_mode=DR)
                    if has_bias:
                        # bias is one value per partition per instruction,
                        # so nonzero b1 needs an ACT per m-tile.
                        for mm in range(MP):
                            m = mp * MP + mm
                            nc.scalar.activation(
                                ht8[:, m, c, :], ph[:, mm, :], TANH,
                                bias=b1_sb[:, m:m + 1], scale=ASCALE)
                    else:
                        nc.scalar.activation(
                            ht8[:, mp * MP:(mp + 1) * MP, c, :],
                            ph[:, :, :], TANH, scale=ASCALE)
                for dm in range(D_T):
                    pz = ps_z_pool.tile([P, NW], F32, tag="ps_z", name="pz")
                    for t in range(KT2):
                        nc.tensor.matmul(
                            pz[:],
                            w2_sb[wi][:, t, :, dm * P:(dm + 1) * P],
                            ht8[:, 2 * t:2 * t + 2, c, :],
                            start=(t == 0), stop=(t == KT2 - 1),
                            perf_mode=DR)
                    consume(c, dm, pz)

            def _stt(out4, c, dm, pz, scal, in4):
                nc.vector.scalar_tensor_tensor(
                    out4[:, dm, c, :], pz[:], scal,
                    in4[:, dm, c, :], op0=MULT, op1=ADD)

            def consume_k1(c, dm, pz):
                # pzb = pz (+ S2*b2) = S2*f(y);  Ymid8 = e4m3(Y + C1*pzb)
                # first (the next sub-feval reads it), then Yacc = Y+CH*pzb.
                if has_bias:
                    nc.vector.tensor_scalar_add(pz[:], pz[:],
                                                b2_sb[:, dm:dm + 1])
                _stt(ym8, c, dm, pz, C1, y_sb)
                _stt(y_acc, c, dm, pz, CH, y_sb)

            def make_consume_k2(last):
                def consume_k2(c, dm, pz):
                    if has_bias:
                        nc.vector.tensor_scalar_add(pz[:], pz[:],
                                                    b2_sb[:, dm:dm + 1])
                    if not last:
                        _stt(y8, c, dm, pz, CH, y_acc)
                    _stt(y_sb, c, dm, pz, CH, y_acc)
                    if last:
                        # overlapped final store of this finished slice
                        nc.sync.dma_start(
                            outt[dm * P:(dm + 1) * P, c * NW:(c + 1) * NW],
                            y_sb[:, dm, c, :])
                return consume_k2

            fev_list = []
            for step in range(N_STEPS):
                fev_list.append((y8, (2 * step) % KW, consume_k1))
                fev_list.append((ym8, (2 * step + 1) % KW,
                                 make_consume_k2(step == N_STEPS - 1)))
            for X8, wi, cons in fev_list:
                for c in range(NCHUNK):
                    sub_feval(X8, wi, c, cons)
            # (final stores are emitted inside the last consume_k2)

    nc.compile()
    return nc


def get_nc(has_bias=False):
    if has_bias not in _NC_CACHE:
        _NC_CACHE[has_bias] = _build(has_bias)
    return _NC_CACHE[has_bias]


def _comp_copies(W, s):
    """KW residual-compensated e4m3 copies of s*W, as uint8 bit patterns."""
    sW = (s * W).astype(np.float32)
    copies, acc = [], np.zeros_like(sW)
    for i in range(KW):
        c = np.clip((i + 1) * sW - acc, -240.0, 240.0) \
            .astype(ml_dtypes.float8_e4m3)
        copies.append(c.view(np.uint8))
        acc += c.astype(np.float32)
    return np.ascontiguousarray(np.stack(copies))


def run(inputs, trace=False, **kwargs):
    y0 = np.asarray(inputs["y0"], dtype=np.float32)
    W1 = np.ascontiguousarray(np.asarray(inputs["W1"], dtype=np.float32))
    b1 = np.ascontiguousarray(np.asarray(inputs["b1"], dtype=np.float32))
    W2 = np.ascontiguousarray(np.asarray(inputs["W2"], dtype=np.float32))
    b2 = np.ascontiguousarray(np.asarray(inputs["b2"], dtype=np.float32))
    nc = get_nc(has_bias=bool(np.any(b1) or np.any(b2)))
    # pre-pack weights partition-major [KW, P, kt*2*cols] so each copy is
    # one contiguous full-bandwidth DMA matching the SBUF tile layout
    w1q = np.ascontiguousarray(
        _comp_copies(W1, S1).reshape(KW, KT1, 2, P, H)
        .transpose(0, 3, 1, 2, 4).reshape(KW, P, KT1 * 2 * H))
    w2q = np.ascontiguousarray(
        _comp_copies(W2, S2).reshape(KW, KT2, 2, P, D)
        .transpose(0, 3, 1, 2, 4).reshape(KW, P, KT2 * 2 * D))
    # biases pre-transposed to [P, tiles] so their DMAs are contiguous
    b1t = np.ascontiguousarray(b1.reshape(H_T, P).T)
    b2st = np.ascontiguousarray((np.float32(S2) * b2).reshape(D_T, P).T)
    # shard over batch, pre-transpose each shard to [D, B] feature-major,
    # pre-scale by SY, then pack partition-major [P, D_T*B];
    # plus the e4m3 bits of the scaled shard.
    shards_t = (np.float32(SY) * y0).reshape(N_CORES, B, D) \
        .transpose(0, 2, 1)                                  # [NC, D, B]
    shards_q = np.clip(shards_t, -240.0, 240.0) \
        .astype(ml_dtypes.float8_e4m3).view(np.uint8)
    shards_t = np.ascontiguousarray(
        shards_t.reshape(N_CORES, D_T, P, B).transpose(0, 2, 1, 3)
        .reshape(N_CORES, P, D_T * B))
    shards_q = np.ascontiguousarray(
        shards_q.reshape(N_CORES, D_T, P, B).transpose(0, 2, 1, 3)
        .reshape(N_CORES, P, D_T * B))
    in_maps = [{"y0t": shards_t[i], "y0q": shards_q[i],
                "w1q": w1q, "b1": b1t, "w2q": w2q, "b2s": b2st}
               for i in range(N_CORES)]
    res = run_bass_kernel_spmd(nc, in_maps, core_ids=list(range(N_CORES)),
                               trace=trace, **kwargs)
    out_t = np.stack([r["outt"] for r in res.results])      # [8, D, B]
    full = np.ascontiguousarray(
        out_t.transpose(0, 2, 1).reshape(BATCH, D) * np.float32(1.0 / SY))
    return full, res


def kernel(**inputs) -> np.ndarray:
    full, _ = run(inputs, trace=False)
    return full
